# revision 1
# baseline (speedup 1.0000x reference)
"""Dropless MoE FFN (router + top-2 dispatch + per-expert MLP + combine) on
8 Trainium2 NeuronCores.

Strategy (expert parallelism, per the sharding hint):
  - Router (softmax + top-2) runs on host in fp32 — it is ~0.02% of the
    FLOPs and IS the token dispatch: each of the 8 cores owns one expert
    and receives only the tokens routed to it (gather on host replaces the
    device all-to-all; full inputs in / full output out per the contract).
  - Each core computes yT = w2_e.T @ gelu_tanh(w1_e.T @ xT) for its ~1k
    routed tokens, entirely in bf16 matmuls (fp32 PSUM accumulation),
    weights read from HBM exactly once.
  - Host applies the combine weights and scatter-adds the two expert
    outputs per token.

Device kernel layout per core (C = token capacity = max routed count):
  Both GEMMs keep tokens on the MOVING (free) dim — GEMM1 produces
  hT[f_tile, tokens] (no transpose between the GEMMs), GEMM2 produces
  yT[d_tile, tokens] — so the ragged token tail only costs short-N
  matmuls, never full tiles.  GELU is applied PSUM->SBUF on ScalarE
  (bf16 hT resident in SBUF).  In GEMM2 the smallest token group runs
  first per weight position so the next position's LDWEIGHTS hides under
  a long N=512 matmul.  Head DMAs are split across both HWDGE rings
  (sync + scalar) and interleaved (w1 chunk, xt chunk) so the PE starts
  within ~2us and ramps while the initial ~3MB streams in; w2 prefetch
  rides the SWDGE ring, dependency-paced behind GEMM1 progress.
"""

import sys

for _p in ("/opt/trn_rl_repo",):
    if _p not in sys.path:
        sys.path.insert(0, _p)

import numpy as np
import ml_dtypes

BF16 = ml_dtypes.bfloat16

D_MODEL = 1024
D_FFN = 4096
N_EXPERTS = 8
TOP_K = 2
N_CORES = 8
P = 128                 # SBUF/PSUM partitions
KC = D_MODEL // P       # 8 contraction chunks for GEMM1
FC = D_FFN // P         # 32 f-chunks (contraction chunks for GEMM2)
MB = 4                  # w1 streamed in 4 blocks of 1024 f-columns

_kernel_cache: dict[int, object] = {}


def _token_groups(C):
    """Split C token columns into <=512-wide PSUM-bank-sized groups,
    as equal as possible (N>=256 keeps the PE issue overhead hidden)."""
    n_g = -(-C // 512)
    base, rem = divmod(C, n_g)
    sizes = [base + (1 if g < rem else 0) for g in range(n_g)]
    groups = []
    off = 0
    for sz in sizes:
        groups.append((off, sz))
        off += sz
    return groups


def _build(C):
    import concourse.bass as bass
    import concourse.mybir as mybir
    import concourse.tile as tile
    from concourse.tile_rust import add_dep_helper
    from concourse import bacc

    dt = mybir.dt
    AF = mybir.ActivationFunctionType
    TT = -(-C // P)                    # token tiles (last may be ragged)
    groups = _token_groups(C)

    nc = bacc.Bacc("TRN2", target_bir_lowering=False, debug=False,
                   num_devices=N_CORES)
    xt_d = nc.dram_tensor("xt", [KC, P, C], dt.bfloat16,
                          kind="ExternalInput").ap()
    w1_d = nc.dram_tensor("w1", [KC, P, D_FFN], dt.bfloat16,
                          kind="ExternalInput").ap()
    w2_d = nc.dram_tensor("w2", [FC // 4, P, 4, D_MODEL], dt.bfloat16,
                          kind="ExternalInput").ap()
    y_d = nc.dram_tensor("y", [KC, P, C], dt.float32,
                         kind="ExternalOutput").ap()

    with tile.TileContext(nc) as tc:
        with (
            tc.tile_pool(name="xt", bufs=KC) as xt_pool,
            tc.tile_pool(name="w1", bufs=2 * KC) as w1_pool,
            tc.tile_pool(name="w2", bufs=FC // 4) as w2_pool,
            tc.tile_pool(name="ht", bufs=FC // 4) as ht_pool,
            tc.tile_pool(name="yo", bufs=2) as y_pool,
            tc.tile_pool(name="ps", bufs=8, space=bass.MemorySpace.PSUM) as ps_pool,
        ):
            # w1 streamed in five f-column blocks; the small first block
            # (4 m-tiles, 1MB) shrinks the critical head bytes so the PE
            # starts sooner.  Later blocks are paced naturally by slot-WAR
            # (bufs = 2 blocks of 8 kc-tiles).
            BLOCKS = [(0, 4), (4, 4), (8, 8), (16, 8), (24, 8)]
            # Head: interleave (w1 block0 chunk -> sync ring, xt chunk ->
            # scalar ring) so GEMM1 m=0 can start as chunks land.
            w1_first = []
            xt_t = []
            for kc in range(KC):
                w = w1_pool.tile([P, BLOCKS[0][1] * P], dt.bfloat16,
                                 tag="w1", name=f"w1_0_{kc}")
                nc.sync.dma_start(w[:], w1_d[kc][:, :BLOCKS[0][1] * P])
                w1_first.append(w)
                t = xt_pool.tile([P, C], dt.bfloat16, tag="xt",
                                 name=f"xt_{kc}")
                nc.scalar.dma_start(t[:], xt_d[kc])
                xt_t.append(t)

            # ---- GEMM1: hT[m*128+p, t] = sum_k w1[k, f] * x[t, k], + GELU
            ht_t = []
            gelu_insts = []
            w1_t = w1_first
            blk_of_m = {}
            for b, (m0, bm) in enumerate(BLOCKS):
                for m in range(m0, m0 + bm):
                    blk_of_m[m] = (b, m0, bm)
            for m in range(FC):
                b, m0, bm = blk_of_m[m]
                mi = m - m0
                if mi == 0 and b > 0:
                    w1_t = [w1_pool.tile([P, bm * P], dt.bfloat16, tag="w1",
                                         name=f"w1_{b}_{kc}")
                            for kc in range(KC)]
                    for kc in range(KC):
                        dma = nc.sync.dma_start(
                            w1_t[kc][:], w1_d[kc][:, m0 * P:(m0 + bm) * P])
                        if b == 1:
                            # block1's slots are free from the start; hold its
                            # 1MB back until the head-critical loads are in
                            add_dep_helper(dma.ins, gelu_insts[0].ins,
                                           sync=True,
                                           reason="pace w1 block1 after head")
                ps = [ps_pool.tile([P, 512], dt.float32, tag="ps1",
                                    name=f"ps1_{m}_{g}")
                      for g in range(len(groups))]
                for kc in range(KC):
                    lhsT = w1_t[kc][:, mi * P:(mi + 1) * P]
                    for g, (off, sz) in enumerate(groups):
                        nc.tensor.matmul(ps[g][:, :sz], lhsT,
                                         xt_t[kc][:, off:off + sz],
                                         start=(kc == 0), stop=(kc == KC - 1))
                if m % 4 == 0:
                    ht = ht_pool.tile([P, 4, C], dt.bfloat16, tag="ht",
                                      name=f"ht_{m // 4}")
                    ht_t.append(ht)
                gelu_inst = None
                for g, (off, sz) in enumerate(groups):
                    gelu_inst = nc.scalar.activation(ht[:, m % 4, off:off + sz],
                                                     ps[g][:, :sz],
                                                     AF.Gelu_apprx_tanh)
                gelu_insts.append(gelu_inst)

            # w2 prefetch on the SWDGE ring (gpsimd is otherwise idle), each
            # chunk paced behind a later GELU so the 8MB of w2 never steals
            # HBM bandwidth from the critical head loads (xt + w1 gate the
            # PE ramp); all chunks land a few iterations before GEMM2 needs
            # them
            w2_t = []
            for j in range(FC // 4):
                w2t = w2_pool.tile([P, 4, D_MODEL], dt.bfloat16, tag="w2",
                                   name=f"w2_{j}")
                w2_dma = nc.gpsimd.dma_start(w2t[:], w2_d[j])
                pace = 6 + (j * 23) // max(FC // 4 - 1, 1)
                add_dep_helper(w2_dma.ins, gelu_insts[pace].ins, sync=True,
                               reason="pace w2 prefetch behind GEMM1 progress")
                w2_t.append(w2t)

            # ---- GEMM2 (flipped): yT[dchunk*128+p, t] = sum_f w2[f, d] * h[t, f]
            # Tokens ride the moving dim, so the ragged tail costs only
            # N=62-cycle matmuls instead of full N=512 ones.  Within each
            # weight position the smallest group goes FIRST so the next
            # position's LDWEIGHTS always hides under a long N=512 matmul.
            # Combine weights are applied on the host (partitions are now D).
            g_order = sorted(range(len(groups)), key=lambda g: groups[g][1])
            for dc in range(KC):
                psg = [ps_pool.tile([P, 512], dt.float32, tag="ps1",
                                    name=f"psy_{dc}_{g}")
                       for g in range(len(groups))]
                for fc in range(FC):
                    lhsT = w2_t[fc // 4][:, fc % 4, dc * P:(dc + 1) * P]
                    for g in g_order:
                        off, sz = groups[g]
                        nc.tensor.matmul(psg[g][:, :sz], lhsT,
                                         ht_t[fc // 4][:, fc % 4, off:off + sz],
                                         start=(fc == 0), stop=(fc == FC - 1))
                y_t = y_pool.tile([P, C], dt.float32, tag="yo")
                for g, (off, sz) in enumerate(groups):
                    nc.scalar.activation(y_t[:, off:off + sz], psg[g][:, :sz],
                                         AF.Copy)
                nc.sync.dma_start(y_d[dc], y_t[:])

    nc.compile()
    return nc


def _route(x, router_w):
    """Replicate the reference router math (jax on CPU, fp32)."""
    import jax
    import jax.numpy as jnp

    with jax.default_device(jax.devices("cpu")[0]):
        xt = jnp.asarray(np.asarray(x, np.float32)).reshape(-1, D_MODEL)
        logits = xt @ jnp.asarray(np.asarray(router_w, np.float32))
        probs = jax.nn.softmax(logits, axis=-1)
        top_p, top_i = jax.lax.top_k(probs, TOP_K)
    return np.asarray(top_p), np.asarray(top_i)


def _run(x, router_w, w1, w2, trace=False):
    from concourse import bass_utils

    x = np.asarray(x, np.float32)
    w1 = np.asarray(w1, np.float32)
    w2 = np.asarray(w2, np.float32)
    B, S, _ = x.shape
    T = B * S
    xt = x.reshape(T, D_MODEL)

    top_p, top_i = _route(x, router_w)

    idxs, wts = [], []
    maxn = 0
    for e in range(N_EXPERTS):
        hit = top_i == e                       # [T, K]
        sel = hit.any(axis=1)
        idx = np.nonzero(sel)[0]
        w = (top_p * hit).sum(axis=1)[sel]     # combine weight per routed token
        idxs.append(idx)
        wts.append(w.astype(np.float32))
        maxn = max(maxn, len(idx))

    C = max(maxn, 2 * P)
    nc = _kernel_cache.get(C)
    if nc is None:
        nc = _build(C)
        _kernel_cache[C] = nc
    TT = -(-C // P)

    in_maps = []
    for e in range(N_EXPERTS):
        n = len(idxs[e])
        xg = np.zeros((C, D_MODEL), np.float32)
        xg[:n] = xt[idxs[e]]
        xtb = np.ascontiguousarray(xg.T).astype(BF16).reshape(KC, P, C)
        w1b = np.ascontiguousarray(w1[e].astype(BF16).reshape(KC, P, D_FFN))
        w2b = np.ascontiguousarray(w2[e].astype(BF16)
                                   .reshape(FC // 4, 4, P, D_MODEL)
                                   .transpose(0, 2, 1, 3))
        in_maps.append({"xt": xtb, "w1": w1b, "w2": w2b})

    res = bass_utils.run_bass_kernel_spmd(
        nc, in_maps, core_ids=list(range(N_CORES)), trace=trace)

    out = np.zeros((T, D_MODEL), np.float32)
    for e in range(N_EXPERTS):
        n = len(idxs[e])
        yt = np.asarray(res.results[e]["y"], np.float32).reshape(D_MODEL, C)
        out[idxs[e]] += wts[e][:, None] * yt.T[:n]
    return out.reshape(B, S, D_MODEL), res


def kernel(**inputs):
    out, _ = _run(inputs["x"], inputs["router_w"], inputs["w1"], inputs["w2"])
    return out



# revision 3
# speedup vs baseline: 1.0193x; 1.0193x over previous
"""Dropless MoE FFN (router + top-2 dispatch + per-expert MLP + combine) on
8 Trainium2 NeuronCores.

Strategy (tensor parallelism over the FFN dim -- perfectly load balanced):
  - Router (softmax + top-2) runs on host in fp32 (~0.02% of FLOPs); the
    token dispatch is a host-side gather: the 8192 (token, expert) pairs
    are sorted by expert into one column-major activation matrix shared
    by all cores.
  - Each core owns a 512-wide slice of the FFN dim of ALL experts
    (column-parallel W1, row-parallel W2).  It computes, for every routed
    column t with expert e(t):
        y_partial[:, t] = w2_e[fslice, :]^T gelu(w1_e[:, fslice]^T x_t)
    This makes the per-core PE work exactly uniform (8192 columns each,
    zero padding), unlike expert-parallelism which pads every core to the
    most-loaded expert.
  - The F-dim partial outputs are summed across cores on the host (the
    host combine/scatter already exists); fp32 partials keep the math
    identical to a single long PSUM accumulation.

Device kernel layout per core:
  Token columns are processed in single-expert chunks of <=512 (PSUM bank
  width).  Expert-block boundaries are baked into the instruction stream
  at build time (compiled per routing signature, cached).  GEMM1 keeps
  tokens on the moving dim (4 f-tile positions x 8 kc accumulation),
  GELU runs PSUM->SBUF on ScalarE, GEMM2 contracts the local 512 f-rows
  (8 d-tile positions x 4 fk accumulation), PSUM->SBUF copies ride the
  otherwise-idle VectorE, and each chunk leaves as one strided DMA.
  DMA rings are purpose-split so slot-WAR pacing never stalls compute
  issue: sync = w1 + xt, gpsimd = w2, vector = y out, scalar = GELU only.
"""

import sys

for _p in ("/opt/trn_rl_repo",):
    if _p not in sys.path:
        sys.path.insert(0, _p)

import numpy as np
import ml_dtypes

BF16 = ml_dtypes.bfloat16

D_MODEL = 1024
D_FFN = 4096
N_EXPERTS = 8
TOP_K = 2
N_CORES = 8
P = 128                 # SBUF/PSUM partitions
KC = D_MODEL // P       # 8 contraction chunks for GEMM1 / d-tiles for GEMM2
FL = D_FFN // N_CORES   # 512 FFN columns owned per core
FLC = FL // P           # 4 local f-tiles

_kernel_cache: dict[tuple, object] = {}


def _token_groups(n, cap=512):
    """Split n token columns into <=cap-wide PSUM-bank-sized groups,
    as equal as possible."""
    n_g = -(-n // cap)
    base, rem = divmod(n, n_g)
    return [base + (1 if g < rem else 0) for g in range(n_g)]


def _make_chunks(counts):
    """Single-expert chunks of <=512 columns covering the expert-sorted
    column order.  The first chunk is shrunk to 256 so the PE can start
    on a small head DMA; chunk sizes are baked into the program."""
    chunks = []
    off = 0
    for e in range(N_EXPERTS):
        n = counts[e]
        if n == 0:
            continue
        sizes = _token_groups(n)
        if not chunks and sizes[0] >= 384:
            sizes = [256, sizes[0] - 256] + sizes[1:]
        for s in sizes:
            chunks.append((e, off, s))
            off += s
    return tuple(chunks)


def _build(chunks):
    import concourse.bass as bass
    import concourse.mybir as mybir
    import concourse.tile as tile
    from concourse import bacc

    dt = mybir.dt
    AF = mybir.ActivationFunctionType
    CT = sum(s for _, _, s in chunks)
    n_ch = len(chunks)

    nc = bacc.Bacc("TRN2", target_bir_lowering=False, debug=False,
                   num_devices=N_CORES)
    xt_d = nc.dram_tensor("xt", [P, KC, CT], dt.bfloat16,
                          kind="ExternalInput").ap()
    w1_d = nc.dram_tensor("w1", [N_EXPERTS, P, KC, FL], dt.bfloat16,
                          kind="ExternalInput").ap()
    w2_d = nc.dram_tensor("w2", [N_EXPERTS, P, FLC, D_MODEL], dt.bfloat16,
                          kind="ExternalInput").ap()
    y_d = nc.dram_tensor("y", [P, KC, CT], dt.float32,
                         kind="ExternalOutput").ap()

    expert_order = []
    for e, _, _ in chunks:
        if e not in expert_order:
            expert_order.append(e)

    with tile.TileContext(nc) as tc:
        with (
            tc.tile_pool(name="w1", bufs=4) as w1_pool,
            tc.tile_pool(name="w2", bufs=3) as w2_pool,
            tc.tile_pool(name="xt", bufs=6) as xt_pool,
            tc.tile_pool(name="ht", bufs=3) as ht_pool,
            tc.tile_pool(name="yo", bufs=3) as y_pool,
            tc.tile_pool(name="ps", bufs=8, space=bass.MemorySpace.PSUM) as ps_pool,
        ):
            # ---- pass 1: all DMAs, in consumption order.  Pool slot-WAR
            # paces each ring automatically; rings carry only DMAs (plus
            # vector copies) so pacing never blocks compute issue.
            w1_t, w2_t = {}, {}
            first_e = chunks[0][0]
            # head-critical loads split by kc so the first matmul starts
            # after ~120KB instead of ~1.7MB
            w1_t[first_e] = w1_pool.tile([P, KC, FL], dt.bfloat16, tag="w1",
                                         name=f"w1_{first_e}")
            xt_head = xt_pool.tile([P, KC, chunks[0][2]], dt.bfloat16,
                                   tag="xt", name="xt_0")
            head_splits = [(0, 1), (1, 3), (4, 4)]
            for k0, kn in head_splits:
                nc.sync.dma_start(w1_t[first_e][:, k0:k0 + kn, :],
                                  w1_d[first_e][:, k0:k0 + kn, :])
                nc.sync.dma_start(xt_head[:, k0:k0 + kn, :],
                                  xt_d[:, k0:k0 + kn, :chunks[0][2]])
            w2_t[first_e] = w2_pool.tile([P, FLC, D_MODEL], dt.bfloat16,
                                         tag="w2", name=f"w2_{first_e}")
            nc.gpsimd.dma_start(w2_t[first_e][:], w2_d[first_e])

            xt_t = [xt_head]
            for ci, (e, off, s) in enumerate(chunks):
                if ci == 0:
                    continue
                if e not in w1_t:
                    w1_t[e] = w1_pool.tile([P, KC, FL], dt.bfloat16,
                                           tag="w1", name=f"w1_{e}")
                    nc.sync.dma_start(w1_t[e][:], w1_d[e])
                    w2_t[e] = w2_pool.tile([P, FLC, D_MODEL], dt.bfloat16,
                                           tag="w2", name=f"w2_{e}")
                    nc.gpsimd.dma_start(w2_t[e][:], w2_d[e])
                t = xt_pool.tile([P, KC, s], dt.bfloat16, tag="xt",
                                 name=f"xt_{ci}")
                nc.sync.dma_start(t[:], xt_d[:, :, off:off + s])
                xt_t.append(t)

            # ---- pass 2: compute, chunk by chunk
            for ci, (e, off, s) in enumerate(chunks):
                xc = xt_t[ci]
                w1c, w2c = w1_t[e], w2_t[e]
                # GEMM1 + GELU: ht[fi*128+p, t] = gelu(sum_k w1[k, f] x[k, t])
                ht = ht_pool.tile([P, FLC, s], dt.bfloat16, tag="ht",
                                  name=f"ht_{ci}")
                for fi in range(FLC):
                    ps = ps_pool.tile([P, 512], dt.float32, tag="ps",
                                      name=f"ps1_{ci}_{fi}")
                    for kc in range(KC):
                        nc.tensor.matmul(ps[:, :s],
                                         w1c[:, kc, fi * P:(fi + 1) * P],
                                         xc[:, kc, :],
                                         start=(kc == 0), stop=(kc == KC - 1))
                    nc.scalar.activation(ht[:, fi, :], ps[:, :s],
                                         AF.Gelu_apprx_tanh)
                # GEMM2: y[dt*128+p, t] = sum_f w2[f, d] ht[f, t]  (local f)
                ysb = y_pool.tile([P, KC, s], dt.float32, tag="yo",
                                  name=f"y_{ci}")
                last = ci == n_ch - 1
                for dtl in range(KC):
                    ps = ps_pool.tile([P, 512], dt.float32, tag="ps",
                                      name=f"ps2_{ci}_{dtl}")
                    for fk in range(FLC):
                        nc.tensor.matmul(ps[:, :s],
                                         w2c[:, fk, dtl * P:(dtl + 1) * P],
                                         ht[:, fk, :],
                                         start=(fk == 0), stop=(fk == FLC - 1))
                    nc.vector.tensor_copy(ysb[:, dtl, :], ps[:, :s])
                    if last:
                        # per-d-tile writeout so the post-PE tail is tiny
                        nc.gpsimd.dma_start(y_d[:, dtl, off:off + s],
                                            ysb[:, dtl, :])
                if not last:
                    nc.gpsimd.dma_start(y_d[:, :, off:off + s], ysb[:])

    nc.compile()
    return nc


def _route(x, router_w):
    """Replicate the reference router math (jax on CPU, fp32)."""
    import jax
    import jax.numpy as jnp

    with jax.default_device(jax.devices("cpu")[0]):
        xt = jnp.asarray(np.asarray(x, np.float32)).reshape(-1, D_MODEL)
        logits = xt @ jnp.asarray(np.asarray(router_w, np.float32))
        probs = jax.nn.softmax(logits, axis=-1)
        top_p, top_i = jax.lax.top_k(probs, TOP_K)
    return np.asarray(top_p), np.asarray(top_i)


def _run(x, router_w, w1, w2, trace=False):
    from concourse import bass_utils

    x = np.asarray(x, np.float32)
    w1 = np.asarray(w1, np.float32)
    w2 = np.asarray(w2, np.float32)
    B, S, _ = x.shape
    T = B * S
    xt = x.reshape(T, D_MODEL)

    top_p, top_i = _route(x, router_w)

    idxs, wts, counts = [], [], []
    for e in range(N_EXPERTS):
        hit = top_i == e                       # [T, K]
        sel = hit.any(axis=1)
        idx = np.nonzero(sel)[0]
        w = (top_p * hit).sum(axis=1)[sel]     # combine weight per routed token
        idxs.append(idx)
        wts.append(w.astype(np.float32))
        counts.append(len(idx))

    chunks = _make_chunks(counts)
    CT = sum(s for _, _, s in chunks)
    nc = _kernel_cache.get(chunks)
    if nc is None:
        nc = _build(chunks)
        _kernel_cache[chunks] = nc

    # expert-sorted gathered activations, [P, KC, CT] (partition = d % 128)
    cols = np.concatenate([idxs[e] for e in range(N_EXPERTS) if counts[e]])
    xg = xt[cols]                                        # [CT, D]
    xtb = np.ascontiguousarray(
        xg.T.reshape(KC, P, CT).transpose(1, 0, 2)).astype(BF16)

    in_maps = []
    for c in range(N_CORES):
        cs = c * FL
        w1b = np.ascontiguousarray(
            w1[:, :, cs:cs + FL].reshape(N_EXPERTS, KC, P, FL)
            .transpose(0, 2, 1, 3)).astype(BF16)
        w2b = np.ascontiguousarray(
            w2[:, cs:cs + FL, :].reshape(N_EXPERTS, FLC, P, D_MODEL)
            .transpose(0, 2, 1, 3)).astype(BF16)
        in_maps.append({"xt": xtb, "w1": w1b, "w2": w2b})

    res = bass_utils.run_bass_kernel_spmd(
        nc, in_maps, core_ids=list(range(N_CORES)), trace=trace)

    # host combine: sum the F-dim partials, then weighted scatter per expert
    ysum = np.zeros((P, KC, CT), np.float32)
    for c in range(N_CORES):
        ysum += np.asarray(res.results[c]["y"], np.float32)
    yfull = ysum.transpose(1, 0, 2).reshape(D_MODEL, CT)  # [D, CT]

    out = np.zeros((T, D_MODEL), np.float32)
    off = 0
    for e in range(N_EXPERTS):
        n = counts[e]
        if n == 0:
            continue
        out[idxs[e]] += wts[e][:, None] * yfull[:, off:off + n].T
        off += n
    return out.reshape(B, S, D_MODEL), res


def kernel(**inputs):
    out, _ = _run(inputs["x"], inputs["router_w"], inputs["w1"], inputs["w2"])
    return out


# revision 6
# speedup vs baseline: 1.0293x; 1.0098x over previous
"""Dropless MoE FFN (router + top-2 dispatch + per-expert MLP + combine) on
8 Trainium2 NeuronCores.

Strategy (tensor parallelism over the FFN dim -- perfectly load balanced):
  - Router (softmax + top-2) runs on host in fp32 (~0.02% of FLOPs); the
    token dispatch is a host-side gather: the 8192 (token, expert) pairs
    are sorted by expert into one column-major activation matrix shared
    by all cores.
  - Each core owns a 512-wide slice of the FFN dim of ALL experts
    (column-parallel W1, row-parallel W2).  It computes, for every routed
    column t with expert e(t):
        y_partial[:, t] = w2_e[fslice, :]^T gelu(w1_e[:, fslice]^T x_t)
    This makes the per-core PE work exactly uniform (8192 columns each,
    zero padding), unlike expert-parallelism which pads every core to the
    most-loaded expert.
  - The F-dim partial outputs are summed across cores on the host (the
    host combine/scatter already exists); fp32 partials keep the math
    identical to a single long PSUM accumulation.

Device kernel layout per core:
  Token columns are processed in single-expert chunks of <=512 (PSUM bank
  width).  Expert-block boundaries are baked into the instruction stream
  at build time (compiled per routing signature, cached).  GEMM1 keeps
  tokens on the moving dim (4 f-tile positions x 8 kc accumulation),
  GELU runs PSUM->SBUF on ScalarE, GEMM2 contracts the local 512 f-rows
  (8 d-tile positions x 4 fk accumulation), PSUM->SBUF copies ride the
  otherwise-idle VectorE, and each chunk leaves as one strided DMA.
  DMA rings are purpose-split so slot-WAR pacing never stalls compute
  issue: sync = w1 + xt, gpsimd = w2, vector = y out, scalar = GELU only.
"""

import sys

for _p in ("/opt/trn_rl_repo",):
    if _p not in sys.path:
        sys.path.insert(0, _p)

import numpy as np
import ml_dtypes

BF16 = ml_dtypes.bfloat16

D_MODEL = 1024
D_FFN = 4096
N_EXPERTS = 8
TOP_K = 2
N_CORES = 8
P = 128                 # SBUF/PSUM partitions
KC = D_MODEL // P       # 8 contraction chunks for GEMM1 / d-tiles for GEMM2
FL = D_FFN // N_CORES   # 512 FFN columns owned per core
FLC = FL // P           # 4 local f-tiles

_kernel_cache: dict[tuple, object] = {}


def _token_groups(n, cap=512):
    """Split n token columns into <=cap-wide PSUM-bank-sized groups,
    as equal as possible."""
    n_g = -(-n // cap)
    base, rem = divmod(n, n_g)
    return [base + (1 if g < rem else 0) for g in range(n_g)]


def _make_chunks(counts):
    """Single-expert chunks of <=512 columns covering the expert-sorted
    column order.  The first chunk is shrunk to 256 so the PE can start
    on a small head DMA; chunk sizes are baked into the program."""
    chunks = []
    off = 0
    for e in range(N_EXPERTS):
        n = counts[e]
        if n == 0:
            continue
        sizes = _token_groups(n)
        if not chunks and sizes[0] >= 384:
            sizes = [256, sizes[0] - 256] + sizes[1:]
        for s in sizes:
            chunks.append((e, off, s))
            off += s
    return tuple(chunks)


def _build(chunks):
    import concourse.bass as bass
    import concourse.mybir as mybir
    import concourse.tile as tile
    from concourse import bacc

    dt = mybir.dt
    AF = mybir.ActivationFunctionType
    CT = sum(s for _, _, s in chunks)
    n_ch = len(chunks)

    nc = bacc.Bacc("TRN2", target_bir_lowering=False, debug=False,
                   num_devices=N_CORES)
    xt_d = nc.dram_tensor("xt", [P, KC, CT], dt.bfloat16,
                          kind="ExternalInput").ap()
    w1_d = nc.dram_tensor("w1", [N_EXPERTS, P, KC, FL], dt.bfloat16,
                          kind="ExternalInput").ap()
    w2_d = nc.dram_tensor("w2", [N_EXPERTS, P, FLC, D_MODEL], dt.bfloat16,
                          kind="ExternalInput").ap()
    y_d = nc.dram_tensor("y", [P, KC, CT], dt.bfloat16,
                         kind="ExternalOutput").ap()

    expert_order = []
    for e, _, _ in chunks:
        if e not in expert_order:
            expert_order.append(e)

    with tile.TileContext(nc) as tc:
        with (
            tc.tile_pool(name="w1", bufs=4) as w1_pool,
            tc.tile_pool(name="w2", bufs=3) as w2_pool,
            tc.tile_pool(name="xt", bufs=6) as xt_pool,
            tc.tile_pool(name="ht", bufs=3) as ht_pool,
            tc.tile_pool(name="yo", bufs=3) as y_pool,
            tc.tile_pool(name="ps", bufs=8, space=bass.MemorySpace.PSUM) as ps_pool,
        ):
            # ---- pass 1: all DMAs, in consumption order.  Pool slot-WAR
            # paces each ring automatically; rings carry only DMAs (plus
            # vector copies) so pacing never blocks compute issue.
            w1_t, w2_t = {}, {}
            first_e = chunks[0][0]
            # head-critical loads split by kc so the first matmul starts
            # after ~120KB instead of ~1.7MB
            w1_t[first_e] = w1_pool.tile([P, KC, FL], dt.bfloat16, tag="w1",
                                         name=f"w1_{first_e}")
            xt_head = xt_pool.tile([P, KC, chunks[0][2]], dt.bfloat16,
                                   tag="xt", name="xt_0")
            head_splits = [(0, 1), (1, 3), (4, 4)]
            for k0, kn in head_splits:
                nc.sync.dma_start(w1_t[first_e][:, k0:k0 + kn, :],
                                  w1_d[first_e][:, k0:k0 + kn, :])
                nc.sync.dma_start(xt_head[:, k0:k0 + kn, :],
                                  xt_d[:, k0:k0 + kn, :chunks[0][2]])
            # first expert's w2 is needed ~5us in; SWDGE cold-start latency
            # is ~15us, so ride the scalar HWDGE ring (idle before GELUs)
            w2_t[first_e] = w2_pool.tile([P, FLC, D_MODEL], dt.bfloat16,
                                         tag="w2", name=f"w2_{first_e}")
            nc.scalar.dma_start(w2_t[first_e][:], w2_d[first_e])

            xt_t = [xt_head]
            for ci, (e, off, s) in enumerate(chunks):
                if ci == 0:
                    continue
                if e not in w1_t:
                    w1_t[e] = w1_pool.tile([P, KC, FL], dt.bfloat16,
                                           tag="w1", name=f"w1_{e}")
                    nc.sync.dma_start(w1_t[e][:], w1_d[e])
                    w2_t[e] = w2_pool.tile([P, FLC, D_MODEL], dt.bfloat16,
                                           tag="w2", name=f"w2_{e}")
                    nc.gpsimd.dma_start(w2_t[e][:], w2_d[e])
                t = xt_pool.tile([P, KC, s], dt.bfloat16, tag="xt",
                                 name=f"xt_{ci}")
                nc.sync.dma_start(t[:], xt_d[:, :, off:off + s])
                xt_t.append(t)

            # ---- pass 2: compute, chunk by chunk
            for ci, (e, off, s) in enumerate(chunks):
                xc = xt_t[ci]
                w1c, w2c = w1_t[e], w2_t[e]
                # GEMM1 + GELU: ht[fi*128+p, t] = gelu(sum_k w1[k, f] x[k, t])
                ht = ht_pool.tile([P, FLC, s], dt.bfloat16, tag="ht",
                                  name=f"ht_{ci}")
                for fi in range(FLC):
                    ps = ps_pool.tile([P, 512], dt.float32, tag="ps",
                                      name=f"ps1_{ci}_{fi}")
                    for kc in range(KC):
                        nc.tensor.matmul(ps[:, :s],
                                         w1c[:, kc, fi * P:(fi + 1) * P],
                                         xc[:, kc, :],
                                         start=(kc == 0), stop=(kc == KC - 1))
                    nc.scalar.activation(ht[:, fi, :], ps[:, :s],
                                         AF.Gelu_apprx_tanh)
                # GEMM2: y[dt*128+p, t] = sum_f w2[f, d] ht[f, t]  (local f)
                ysb = y_pool.tile([P, KC, s], dt.bfloat16, tag="yo",
                                  name=f"y_{ci}")
                tailing = ci >= n_ch - 2
                for dtl in range(KC):
                    ps = ps_pool.tile([P, 512], dt.float32, tag="ps",
                                      name=f"ps2_{ci}_{dtl}")
                    for fk in range(FLC):
                        nc.tensor.matmul(ps[:, :s],
                                         w2c[:, fk, dtl * P:(dtl + 1) * P],
                                         ht[:, fk, :],
                                         start=(fk == 0), stop=(fk == FLC - 1))
                    nc.vector.tensor_copy(ysb[:, dtl, :], ps[:, :s])
                    if tailing:
                        # per-d-tile writeout on the (now idle, fast-issue)
                        # sync HWDGE ring so the post-PE tail is tiny
                        nc.sync.dma_start(y_d[:, dtl, off:off + s],
                                          ysb[:, dtl, :])
                if not tailing:
                    nc.gpsimd.dma_start(y_d[:, :, off:off + s], ysb[:])

    nc.compile()
    return nc


def _route(x, router_w):
    """Replicate the reference router math (jax on CPU, fp32)."""
    import jax
    import jax.numpy as jnp

    with jax.default_device(jax.devices("cpu")[0]):
        xt = jnp.asarray(np.asarray(x, np.float32)).reshape(-1, D_MODEL)
        logits = xt @ jnp.asarray(np.asarray(router_w, np.float32))
        probs = jax.nn.softmax(logits, axis=-1)
        top_p, top_i = jax.lax.top_k(probs, TOP_K)
    return np.asarray(top_p), np.asarray(top_i)


def _run(x, router_w, w1, w2, trace=False):
    from concourse import bass_utils

    x = np.asarray(x, np.float32)
    w1 = np.asarray(w1, np.float32)
    w2 = np.asarray(w2, np.float32)
    B, S, _ = x.shape
    T = B * S
    xt = x.reshape(T, D_MODEL)

    top_p, top_i = _route(x, router_w)

    idxs, wts, counts = [], [], []
    for e in range(N_EXPERTS):
        hit = top_i == e                       # [T, K]
        sel = hit.any(axis=1)
        idx = np.nonzero(sel)[0]
        w = (top_p * hit).sum(axis=1)[sel]     # combine weight per routed token
        idxs.append(idx)
        wts.append(w.astype(np.float32))
        counts.append(len(idx))

    chunks = _make_chunks(counts)
    CT = sum(s for _, _, s in chunks)
    nc = _kernel_cache.get(chunks)
    if nc is None:
        nc = _build(chunks)
        _kernel_cache[chunks] = nc

    # expert-sorted gathered activations, [P, KC, CT] (partition = d % 128)
    cols = np.concatenate([idxs[e] for e in range(N_EXPERTS) if counts[e]])
    xg = xt[cols]                                        # [CT, D]
    xtb = np.ascontiguousarray(
        xg.T.reshape(KC, P, CT).transpose(1, 0, 2)).astype(BF16)

    in_maps = []
    for c in range(N_CORES):
        cs = c * FL
        w1b = np.ascontiguousarray(
            w1[:, :, cs:cs + FL].reshape(N_EXPERTS, KC, P, FL)
            .transpose(0, 2, 1, 3)).astype(BF16)
        w2b = np.ascontiguousarray(
            w2[:, cs:cs + FL, :].reshape(N_EXPERTS, FLC, P, D_MODEL)
            .transpose(0, 2, 1, 3)).astype(BF16)
        in_maps.append({"xt": xtb, "w1": w1b, "w2": w2b})

    res = bass_utils.run_bass_kernel_spmd(
        nc, in_maps, core_ids=list(range(N_CORES)), trace=trace)

    # host combine: sum the F-dim partials, then weighted scatter per expert
    ysum = np.zeros((P, KC, CT), np.float32)
    for c in range(N_CORES):
        ysum += np.asarray(res.results[c]["y"], np.float32)
    yfull = ysum.transpose(1, 0, 2).reshape(D_MODEL, CT)  # [D, CT]

    out = np.zeros((T, D_MODEL), np.float32)
    off = 0
    for e in range(N_EXPERTS):
        n = counts[e]
        if n == 0:
            continue
        out[idxs[e]] += wts[e][:, None] * yfull[:, off:off + n].T
        off += n
    return out.reshape(B, S, D_MODEL), res


def kernel(**inputs):
    out, _ = _run(inputs["x"], inputs["router_w"], inputs["w1"], inputs["w2"])
    return out


# revision 11
# speedup vs baseline: 1.0452x; 1.0155x over previous
"""Dropless MoE FFN (router + top-2 dispatch + per-expert MLP + combine) on
8 Trainium2 NeuronCores.

Strategy (tensor parallelism over the FFN dim -- perfectly load balanced):
  - Router (softmax + top-2) runs on host in fp32 (~0.02% of FLOPs); the
    token dispatch is a host-side gather: the 8192 (token, expert) pairs
    are sorted by expert into one column-major activation matrix shared
    by all cores.
  - Each core owns a 512-wide slice of the FFN dim of ALL experts
    (column-parallel W1, row-parallel W2).  It computes, for every routed
    column t with expert e(t):
        y_partial[:, t] = w2_e[fslice, :]^T gelu(w1_e[:, fslice]^T x_t)
    This makes the per-core PE work exactly uniform (8192 columns each,
    zero padding), unlike expert-parallelism which pads every core to the
    most-loaded expert.
  - The F-dim partial outputs are summed across cores on the host (the
    host combine/scatter already exists); fp32 partials keep the math
    identical to a single long PSUM accumulation.

Device kernel layout per core:
  Token columns are processed in single-expert chunks of <=512 (PSUM bank
  width).  Expert-block boundaries are baked into the instruction stream
  at build time (compiled per routing signature, cached).  GEMM1 keeps
  tokens on the moving dim (4 f-tile positions x 8 kc accumulation),
  GELU runs PSUM->SBUF on ScalarE, GEMM2 contracts the local 512 f-rows
  (8 d-tile positions x 4 fk accumulation), PSUM->SBUF copies ride the
  otherwise-idle VectorE, and each chunk leaves as one strided DMA.
  DMA rings are purpose-split so slot-WAR pacing never stalls compute
  issue: sync = w1 + xt, gpsimd = w2, vector = y out, scalar = GELU only.
"""

import sys

for _p in ("/opt/trn_rl_repo",):
    if _p not in sys.path:
        sys.path.insert(0, _p)

import numpy as np
import ml_dtypes

BF16 = ml_dtypes.bfloat16

D_MODEL = 1024
D_FFN = 4096
N_EXPERTS = 8
TOP_K = 2
N_CORES = 8
P = 128                 # SBUF/PSUM partitions
KC = D_MODEL // P       # 8 contraction chunks for GEMM1 / d-tiles for GEMM2
FL = D_FFN // N_CORES   # 512 FFN columns owned per core
FLC = FL // P           # 4 local f-tiles

_kernel_cache: dict[tuple, object] = {}


def _token_groups(n, cap=512):
    """Split n token columns into <=cap-wide PSUM-bank-sized groups,
    as equal as possible."""
    n_g = -(-n // cap)
    base, rem = divmod(n, n_g)
    return [base + (1 if g < rem else 0) for g in range(n_g)]


def _make_chunks(counts):
    """Single-expert chunks of <=512 columns covering the expert-sorted
    column order.  The first chunk is shrunk to 256 so the PE can start
    on a small head DMA; chunk sizes are baked into the program."""
    chunks = []
    off = 0
    for e in range(N_EXPERTS):
        n = counts[e]
        if n == 0:
            continue
        sizes = _token_groups(n)
        if not chunks and sizes[0] >= 384:
            sizes = [256, sizes[0] - 256] + sizes[1:]
        for s in sizes:
            chunks.append((e, off, s))
            off += s
    return tuple(chunks)


def _build(chunks):
    import concourse.bass as bass
    import concourse.mybir as mybir
    import concourse.tile as tile
    from concourse import bacc

    dt = mybir.dt
    AF = mybir.ActivationFunctionType
    CT = sum(s for _, _, s in chunks)
    n_ch = len(chunks)

    nc = bacc.Bacc("TRN2", target_bir_lowering=False, debug=False,
                   num_devices=N_CORES)
    xt_d = nc.dram_tensor("xt", [P, KC, CT], dt.bfloat16,
                          kind="ExternalInput").ap()
    w1_d = nc.dram_tensor("w1", [N_EXPERTS, P, KC, FL], dt.bfloat16,
                          kind="ExternalInput").ap()
    w2_d = nc.dram_tensor("w2", [N_EXPERTS, P, FLC, D_MODEL], dt.bfloat16,
                          kind="ExternalInput").ap()
    y_d = nc.dram_tensor("y", [P, KC, CT], dt.bfloat16,
                         kind="ExternalOutput").ap()

    expert_order = []
    for e, _, _ in chunks:
        if e not in expert_order:
            expert_order.append(e)

    with tile.TileContext(nc) as tc:
        with (
            tc.tile_pool(name="w1", bufs=5) as w1_pool,
            tc.tile_pool(name="w2", bufs=3) as w2_pool,
            tc.tile_pool(name="xt", bufs=7) as xt_pool,
            tc.tile_pool(name="ht", bufs=3) as ht_pool,
            tc.tile_pool(name="yo", bufs=3) as y_pool,
            tc.tile_pool(name="ps", bufs=8, space=bass.MemorySpace.PSUM) as ps_pool,
        ):
            # ---- pass 1: all DMAs, in consumption order.  Pool slot-WAR
            # paces each ring automatically; rings carry only DMAs (plus
            # vector copies) so pacing never blocks compute issue.
            w1_t, w2_t = {}, {}
            first_e = chunks[0][0]
            s0 = chunks[0][2]
            # head-critical loads as SEPARATE TILES per kc-band so the RAW
            # deps are fine-grained: the first matmul starts after ~210KB
            # (kc0 band) instead of the whole 1.7MB
            head_bands = [(0, 1), (1, 3), (4, 4)]
            w1_head, xt_head = [], []
            for k0, kn in head_bands:
                wt = w1_pool.tile([P, kn, FL], dt.bfloat16, tag="w1h",
                                  name=f"w1h_{k0}")
                nc.sync.dma_start(wt[:], w1_d[first_e][:, k0:k0 + kn, :])
                xh = xt_pool.tile([P, kn, s0], dt.bfloat16, tag="xth",
                                  name=f"xth_{k0}")
                nc.sync.dma_start(xh[:], xt_d[:, k0:k0 + kn, :s0])
                for i in range(kn):
                    w1_head.append((wt, i))
                    xt_head.append((xh, i))
            # first expert's w2 is needed right after chunk0's GEMM1
            # (~5us in): sync ring, ahead of the xt prefetch stream (SWDGE
            # cold-start is ~15us, far too slow for this one)
            w2_t[first_e] = w2_pool.tile([P, FLC, D_MODEL], dt.bfloat16,
                                         tag="w2", name=f"w2_{first_e}")
            nc.sync.dma_start(w2_t[first_e][:], w2_d[first_e])
            # the head bands ARE w1[first_e]'s storage for all its chunks
            w1_t[first_e] = lambda kc, lo, hi, _w=w1_head: \
                _w[kc][0][:, _w[kc][1], lo:hi]

            xt_t = [lambda kc, _x=xt_head: _x[kc][0][:, _x[kc][1], :]]
            for ci, (e, off, s) in enumerate(chunks):
                if ci == 0:
                    continue
                if e not in w1_t:
                    wt = w1_pool.tile([P, KC, FL], dt.bfloat16,
                                      tag="w1", name=f"w1_{e}")
                    nc.sync.dma_start(wt[:], w1_d[e])
                    w1_t[e] = lambda kc, lo, hi, _w=wt: _w[:, kc, lo:hi]
                    w2_t[e] = w2_pool.tile([P, FLC, D_MODEL], dt.bfloat16,
                                           tag="w2", name=f"w2_{e}")
                    nc.gpsimd.dma_start(w2_t[e][:], w2_d[e])
                t = xt_pool.tile([P, KC, s], dt.bfloat16, tag="xt",
                                 name=f"xt_{ci}")
                nc.sync.dma_start(t[:], xt_d[:, :, off:off + s])
                xt_t.append(lambda kc, _t=t: _t[:, kc, :])

            # ---- pass 2: compute, chunk by chunk
            for ci, (e, off, s) in enumerate(chunks):
                xc = xt_t[ci]
                w1c, w2c = w1_t[e], w2_t[e]
                # GEMM1 + GELU: ht[fi*128+p, t] = gelu(sum_k w1[k, f] x[k, t])
                ht = ht_pool.tile([P, FLC, s], dt.bfloat16, tag="ht",
                                  name=f"ht_{ci}")
                for fi in range(FLC):
                    ps = ps_pool.tile([P, 512], dt.float32, tag="ps",
                                      name=f"ps1_{ci}_{fi}")
                    for kc in range(KC):
                        nc.tensor.matmul(ps[:, :s],
                                         w1c(kc, fi * P, (fi + 1) * P),
                                         xc(kc),
                                         start=(kc == 0), stop=(kc == KC - 1))
                    nc.scalar.activation(ht[:, fi, :], ps[:, :s],
                                         AF.Gelu_apprx_tanh)
                # GEMM2: y[dt*128+p, t] = sum_f w2[f, d] ht[f, t]  (local f)
                ysb = y_pool.tile([P, KC, s], dt.bfloat16, tag="yo",
                                  name=f"y_{ci}")
                tailing = ci >= n_ch - 2
                for dtl in range(KC):
                    ps = ps_pool.tile([P, 512], dt.float32, tag="ps",
                                      name=f"ps2_{ci}_{dtl}")
                    for fk in range(FLC):
                        nc.tensor.matmul(ps[:, :s],
                                         w2c[:, fk, dtl * P:(dtl + 1) * P],
                                         ht[:, fk, :],
                                         start=(fk == 0), stop=(fk == FLC - 1))
                    nc.vector.tensor_copy(ysb[:, dtl, :], ps[:, :s])
                    if tailing:
                        # per-d-tile writeout on the (now idle, fast-issue)
                        # sync HWDGE ring so the post-PE tail is tiny
                        nc.sync.dma_start(y_d[:, dtl, off:off + s],
                                          ysb[:, dtl, :])
                if not tailing:
                    nc.gpsimd.dma_start(y_d[:, :, off:off + s], ysb[:])

    nc.compile()
    return nc


def _route(x, router_w):
    """Replicate the reference router math (jax on CPU, fp32)."""
    import jax
    import jax.numpy as jnp

    with jax.default_device(jax.devices("cpu")[0]):
        xt = jnp.asarray(np.asarray(x, np.float32)).reshape(-1, D_MODEL)
        logits = xt @ jnp.asarray(np.asarray(router_w, np.float32))
        probs = jax.nn.softmax(logits, axis=-1)
        top_p, top_i = jax.lax.top_k(probs, TOP_K)
    return np.asarray(top_p), np.asarray(top_i)


def _run(x, router_w, w1, w2, trace=False):
    from concourse import bass_utils

    x = np.asarray(x, np.float32)
    w1 = np.asarray(w1, np.float32)
    w2 = np.asarray(w2, np.float32)
    B, S, _ = x.shape
    T = B * S
    xt = x.reshape(T, D_MODEL)

    top_p, top_i = _route(x, router_w)

    idxs, wts, counts = [], [], []
    for e in range(N_EXPERTS):
        hit = top_i == e                       # [T, K]
        sel = hit.any(axis=1)
        idx = np.nonzero(sel)[0]
        w = (top_p * hit).sum(axis=1)[sel]     # combine weight per routed token
        idxs.append(idx)
        wts.append(w.astype(np.float32))
        counts.append(len(idx))

    chunks = _make_chunks(counts)
    CT = sum(s for _, _, s in chunks)
    nc = _kernel_cache.get(chunks)
    if nc is None:
        nc = _build(chunks)
        _kernel_cache[chunks] = nc

    # expert-sorted gathered activations, [P, KC, CT] (partition = d % 128)
    cols = np.concatenate([idxs[e] for e in range(N_EXPERTS) if counts[e]])
    xg = xt[cols]                                        # [CT, D]
    xtb = np.ascontiguousarray(
        xg.T.reshape(KC, P, CT).transpose(1, 0, 2)).astype(BF16)

    in_maps = []
    for c in range(N_CORES):
        cs = c * FL
        w1b = np.ascontiguousarray(
            w1[:, :, cs:cs + FL].reshape(N_EXPERTS, KC, P, FL)
            .transpose(0, 2, 1, 3)).astype(BF16)
        w2b = np.ascontiguousarray(
            w2[:, cs:cs + FL, :].reshape(N_EXPERTS, FLC, P, D_MODEL)
            .transpose(0, 2, 1, 3)).astype(BF16)
        in_maps.append({"xt": xtb, "w1": w1b, "w2": w2b})

    res = bass_utils.run_bass_kernel_spmd(
        nc, in_maps, core_ids=list(range(N_CORES)), trace=trace)

    # host combine: sum the F-dim partials, then weighted scatter per expert
    ysum = np.zeros((P, KC, CT), np.float32)
    for c in range(N_CORES):
        ysum += np.asarray(res.results[c]["y"], np.float32)
    yfull = ysum.transpose(1, 0, 2).reshape(D_MODEL, CT)  # [D, CT]

    out = np.zeros((T, D_MODEL), np.float32)
    off = 0
    for e in range(N_EXPERTS):
        n = counts[e]
        if n == 0:
            continue
        out[idxs[e]] += wts[e][:, None] * yfull[:, off:off + n].T
        off += n
    return out.reshape(B, S, D_MODEL), res


def kernel(**inputs):
    out, _ = _run(inputs["x"], inputs["router_w"], inputs["w1"], inputs["w2"])
    return out
